# revision 49
# baseline (speedup 1.0000x reference)
"""Gated Linear Attention on 8 Trainium2 NeuronCores.

Sharding: one (batch, head) pair per core (B=2 x H=4 = 8 cores). The recurrent
state is independent per (batch, head); each core computes its head's full
pipeline (projections -> chunked GLA scan -> RMS-norm scale -> silu gate ->
output projection) and emits a partial [N, D] output; the host sums the 4 head
partials per batch.

All matmul operands are bf16 (fp32 matmul costs 4 cycles/row on the PE vs 1
for bf16); PSUM accumulation stays fp32, as do the decay cumsum carry and all
activation intermediates. The activation-table map is patched at compile time
so every ACT function (exp/ln/square/copy) resolves to the one table set that
holds them all ("natural_log_exp_and_others") - the default greedy placement
flip-flops exp_and_others <-> natural_log, costing 33 x 1.28us in reloads.

Device algorithm (chunked, chunk C=128):
  g'' = softplus(-(x@Wz + bgk2))       (= -16*log-decay; the reference clamp
                                        at 48 can never trigger for |z|<~1)
  bT[dk,t] = g''^T @ triu  (intra-chunk cumsum; the cross-chunk carry rides
                            the per-partition ACT bias of the exps below)
  ET = exp(-(bT+carry)/16), EnT = exp(+(bT+carry)/16)
  q~T = qT*ET, k~T = kT*EnT            (q/k projected weight-stationary, so
                                        they come out of PSUM feature-major)
  at[s,t] = k~T^T q~T masked s<=t
  oT[dv,t] = v^T at + W^T q~T          (W[dk,dv] accumulates in PSUM across
                                        chunks; k~ re-transposed on the PE
                                        for the W += k~^T v update)
  og = oT * (u * 1/(1+exp(-u)))        (silu gate; u evicted to SBUF once so
                                        the chain runs off the PSUM bank)
  out[t,:] = r[t] * (og^T @ (rms_w*Wout_head))   (r = rms rsqrt, from an ACT
                                        square + PE column-sum; the r-scale
                                        is fused into the PSUM evictions)

The emission is software-pipelined two chunks deep - iteration i issues chunk
i's projections and decay chain, chunk i-1's attention, and chunk i-2's
state/output stages - so the in-order PE never waits on an intra-chunk
ACT/DVE chain; PSUM banks: pA 1 + pB 1 + at/o 2 + k-transpose 1 + fin 2 +
state 1 = 8, with the last chunk's fin stealing the then-idle projection
banks. Host folds Wgk1@Wgk2 -> Wz and rms_w into Wout; x is shipped
pre-transposed bf16, with chunk 0 + Wz + bgk2 packed into one leading DMA.
"""

import os
from contextlib import ExitStack

import numpy as np

import concourse.bass as bass
import concourse.tile as tile
from concourse import bacc, mybir
from concourse.tile_rust import add_dep_helper
from concourse.bass_utils import run_bass_kernel_spmd

F32 = mybir.dt.float32
BF16 = mybir.dt.bfloat16
AF = mybir.ActivationFunctionType

B, N, D, H = 2, 1024, 1024, 4
KD, VD, DK, DV = 512, 1024, 128, 256
C = 128                    # chunk length (= token partitions)
NCH = N // C               # 8 chunks
NK = D // 128              # 8 contraction tiles
EPS = 1e-5

# module-level stash so test.py can grab profiling results
LAST_RESULTS = None
LAST_NC = None


def _patch_act_tables():
    """Force every activation func used here (Exp/Ln/Square/Copy) to resolve
    to the one table set containing them all, so the compile-time table-load
    pass emits a single LoadActFuncSet instead of ping-ponging between
    exp_and_others and natural_log (33 loads x ~1.28us). Names/order (and
    hence act_func_set_ids) are preserved."""
    import concourse.bacc as bacc_mod
    from concourse.hw_specs import get_activation_tables

    if getattr(bacc_mod, "_gla_act_patch", False):
        return
    orig = get_activation_tables  # functools.cached on arch

    def patched(arch):
        tables = orig(arch)
        common = "natural_log_exp_and_others"
        if common not in tables:
            return tables
        keep = tables[common]
        return {
            name: (funcs if name == common else funcs - keep)
            for name, funcs in tables.items()
        }

    bacc_mod.get_activation_tables = patched
    bacc_mod._gla_act_patch = True


def _emit_kernel(ctx: ExitStack, tc: "tile.TileContext", ap: dict):
    nc = tc.nc

    # Chain all PE instructions in program order. PE executes in-order anyway,
    # but the Tile scheduler may otherwise reorder range-disjoint matmuls
    # within a PSUM bank, breaking has_written clear ordering (start=True
    # marks the whole bank's zero regions pending).
    pe_prev = [None]

    def mm(*args, **kw):
        inst = nc.tensor.matmul(*args, **kw)
        if pe_prev[0] is not None:
            add_dep_helper(inst.ins, pe_prev[0], sync=False, reason="pe-order")
        pe_prev[0] = inst.ins
        return inst

    def tr_(out, in_, ident):
        inst = nc.tensor.transpose(out, in_, ident)
        if pe_prev[0] is not None:
            add_dep_helper(inst.ins, pe_prev[0], sync=False, reason="pe-order")
        pe_prev[0] = inst.ins
        return inst

    xT, wqk, wgv, woutT = ap["xT"], ap["wqk"], ap["wgv"], ap["woutT"]
    x0z = ap["x0z"]
    lmask, identb, out = ap["lmask"], ap["identb"], ap["out"]

    consts = ctx.enter_context(tc.tile_pool(name="consts", bufs=1))
    wpool = ctx.enter_context(tc.tile_pool(name="wpool", bufs=1))
    work = ctx.enter_context(tc.tile_pool(name="work", bufs=2))
    wide = ctx.enter_context(tc.tile_pool(name="wide", bufs=2))
    wst = ctx.enter_context(tc.tile_pool(name="wst", bufs=2))
    pjA = ctx.enter_context(tc.tile_pool(name="pjA", bufs=1, space="PSUM"))
    pjB = ctx.enter_context(tc.tile_pool(name="pjB", bufs=1, space="PSUM"))
    ato = ctx.enter_context(tc.tile_pool(name="ato", bufs=2, space="PSUM"))
    trp = ctx.enter_context(tc.tile_pool(name="trp", bufs=1, space="PSUM"))
    finp = ctx.enter_context(tc.tile_pool(name="finp", bufs=2, space="PSUM"))
    wp = ctx.enter_context(tc.tile_pool(name="wp", bufs=1, space="PSUM"))
    osbp = ctx.enter_context(tc.tile_pool(name="osbp", bufs=2))

    # ---- input DMAs, ordered by first use (payloads serialize on the
    # DMA engines, so order = data-arrival order) ----
    xsb = wpool.tile([128, NK, N], BF16)
    wsb = wpool.tile([128, NK, 896], BF16)   # z 0:128 | qk 128:384 | gv 384:896
    x0z_sb = wpool.tile([128, NK * 128 + NK * 128 + 128], BF16)
    nc.sync.dma_start(out=x0z_sb[:], in_=x0z[:])
    x0_sb = x0z_sb[:, 0:NK * 128]
    bg_sb = x0z_sb[0:1, 2 * NK * 128:2 * NK * 128 + 128]
    nc.sync.dma_start(out=wsb[:, :, 128:384], in_=wqk[:])
    L_sb = consts.tile([128, 128], F32)          # L[s,t]=1 iff s<=t (triu)
    nc.sync.dma_start(out=L_sb[:], in_=lmask[:])
    nc.sync.dma_start(out=wsb[:, :, 384:640], in_=wgv[:, :, 0:256])
    nc.sync.dma_start(out=xsb[:, :, C:3 * C], in_=xT[:, :, C:3 * C])
    nc.sync.dma_start(out=wsb[:, :, 640:896], in_=wgv[:, :, 256:512])
    id_sb = consts.tile([128, 128], BF16)
    nc.sync.dma_start(out=id_sb[:], in_=identb[:])
    wout_sb = wpool.tile([128, 2, D], BF16)
    nc.sync.dma_start(out=wout_sb[:], in_=woutT[:])
    nc.sync.dma_start(out=xsb[:, :, 3 * C:5 * C], in_=xT[:, :, 3 * C:5 * C])
    nc.sync.dma_start(out=xsb[:, :, 5 * C:N], in_=xT[:, :, 5 * C:N])

    # ---- constants from memset ----
    Lb_sb = consts.tile([128, 128], BF16)
    nc.vector.tensor_copy(Lb_sb[:], L_sb[:])
    ones_row_b = consts.tile([1, 128], BF16)
    nc.vector.memset(ones_row_b[:], 1.0)
    fzero = consts.tile([128, 1], F32)
    nc.vector.memset(fzero[:], 0.0)
    ones_col_b = consts.tile([128, 1], BF16)
    nc.vector.memset(ones_col_b[:], 1.0)
    eps_sb = consts.tile([128, 1], F32)
    nc.vector.memset(eps_sb[:], EPS)
    inv_dv = consts.tile([128, 1], BF16)
    nc.vector.memset(inv_dv[:], 1.0 / DV)

    # persistent PSUM bank: cols 0:256 = state W accumulator, cols 256:384 =
    # cumsum carry row (row 0 only). Single never-closed accumulation group.
    wps = wp.tile([128, 257], F32, tag="wps")


    # Software-pipelined emission, 2 chunks deep: iteration i emits chunk
    # i's projections + decay chain (A/B), chunk i-1's attention (D1/D2),
    # and chunk i-2's state/fin/rms/output (C). PE executes in emission
    # order (pe-order chain), so every matmul's vector-engine inputs were
    # produced 1-2 stages earlier and PE never waits on an intra-chunk
    # ACT/DVE chain.
    S = [dict() for _ in range(NCH)]

    for i in range(NCH + 2):
        m1, m2 = i - 1, i - 2

        if i < NCH:
            st = S[i]
            tok = slice(i * C, (i + 1) * C)
            # ---- A1: pB bank: z 0:128 | uT (gate) 128:384 | bT 384:512.
            # bias-mm start=True marks the bank pending-zero; gate-mms are
            # ordinary in-group accumulators; the last z-mm closes the
            # group. bT-mms are late writers (skip_group_check). For chunk
            # 0 the gate-mms go last (skip_group_check) so the z chain is
            # not gated on the gate-weights DMA.
            pB = pjB.tile([128, 512], F32, tag="pb")
            st["pB"] = pB

            def _xk(k):
                return (x0_sb[:, k * 128:(k + 1) * 128] if i == 0
                        else xsb[:, k, tok])

            def _gate_mms(skip):
                for k in range(NK):
                    for half in range(2):
                        gcols = slice(128 + half * 128, 256 + half * 128)
                        wcols = slice(384 + half * 128, 512 + half * 128)
                        mm(pB[:, gcols], lhsT=wsb[:, k, wcols],
                           rhs=_xk(k), start=False, stop=False,
                           skip_group_check=skip)

            if i > 0:
                mm(pB[:, 0:128], lhsT=ones_row_b[:], rhs=bg_sb,
                   start=True, stop=False)
                _gate_mms(False)
                for k in range(NK):
                    mm(pB[:, 0:128], lhsT=_xk(k), rhs=x0z_sb[:, NK * 128 + k * 128:NK * 128 + (k + 1) * 128],
                       start=False, stop=(k == NK - 1))
            else:
                # chunk 0: z-mms lead (start=True) so nothing waits on the
                # bias/gate-weight DMAs; bias closes the group.
                for k in range(NK):
                    mm(pB[:, 0:128], lhsT=_xk(k), rhs=x0z_sb[:, NK * 128 + k * 128:NK * 128 + (k + 1) * 128],
                       start=(k == 0), stop=False)
                mm(pB[:, 0:128], lhsT=ones_row_b[:], rhs=bg_sb,
                   start=False, stop=True)
                _gate_mms(True)
            e1 = work.tile([128, 128], F32, tag="e1")
            nc.scalar.activation(e1[:], pB[:, 0:128], AF.Exp, scale=-1.0)
            g_tm = work.tile([128, 128], BF16, tag="g")
            nc.scalar.activation(g_tm[:], e1[:], AF.Ln, bias=1.0)
            st["g"] = g_tm

        if 0 <= m1 < NCH:
            st1 = S[m1]
            # ---- D1: attention scores + k~ transpose (inputs from i-1) ---
            ao = ato.tile([128, 385], F32, tag="ao")
            st1["ao"] = ao
            mm(ao[:, 0:128], lhsT=st1["kt"][:], rhs=st1["qt"][:],
               start=True, stop=True)
            if m1 < NCH - 1:
                tr_t = trp.tile([128, 128], BF16, tag="tr")
                tr_(tr_t[:], st1["kt"][:], id_sb[:])
            at_m = work.tile([128, 128], BF16, tag="atm")
            nc.vector.tensor_tensor(at_m[:], ao[:, 0:128], L_sb[:],
                                    mybir.AluOpType.mult)
            st1["atm"] = at_m
            st1["trt"] = tr_t if m1 < NCH - 1 else None

        if 0 <= m2 < NCH - 1:
            # ---- C1: state update (before B so the wps tile-level order
            # is state-mm -> this iter's colsum/boff) --------------------
            st2 = S[m2]
            mm(wps[:, 0:256], lhsT=st2["ktm"][:], rhs=st2["v"][:],
               start=False, stop=False, skip_group_check=True)
            w_new = wst.tile([128, DV], BF16, tag="wsb")
            nc.scalar.copy(w_new[:], wps[:, 0:256])
            st2["w"] = w_new

        if i < NCH:
            st = S[i]
            # ---- A2: qT/kT (weight-stationary) + v (x-stationary) --------
            pA = pjA.tile([128, 512], F32, tag="pa")
            st["pA"] = pA
            for k in range(NK):
                mm(pA[:, 0:128], lhsT=wsb[:, k, 128:256], rhs=_xk(k),
                   start=(k == 0), stop=False)
            for k in range(NK):
                mm(pA[:, 128:256], lhsT=wsb[:, k, 256:384],
                   rhs=_xk(k), start=False, stop=(k == NK - 1))
            for k in range(NK):
                mm(pA[:, 256:512], lhsT=_xk(k), rhs=wsb[:, k, 640:896],
                   start=False, stop=False, skip_group_check=True)

            # evict u once so the silu chain runs SBUF-only (off the psum
            # bank, mostly on the otherwise-idle Pool engine)
            u_sb = wide.tile([128, DV], F32, tag="u")
            nc.vector.tensor_copy(u_sb[:], pB[:, 128:384])
            st["u"] = u_sb
            v_tm = wide.tile([128, DV], BF16, tag="v", bufs=3)
            nc.vector.tensor_copy(v_tm[:], pA[:, 256:512])
            st["v"] = v_tm
            # carry accumulation: wps col 256 += colsum(g'') as a [dk,1]
            # column (emitted before C1's w-evict so the wps tile-order is
            # colsum -> w-evict)
            if i < NCH - 1:
                mm(wps[:, 256:257], lhsT=st["g"][:], rhs=ones_col_b[:],
                   start=(i == 0), stop=False, skip_group_check=True)

        if i < NCH:
            st = S[i]
            # ---- B: cumsum bT = g^T L (+ carry bcast), carry colsum ------
            mm(pB[:, 384:512], lhsT=st["g"][:], rhs=Lb_sb[:],
               start=False, stop=False, skip_group_check=True)
            # carry enters as the per-partition ACT bias: exp(-+(bT+boff)/16)
            bn = S[m1]["bn"] if i > 0 else fzero
            bp = S[m1]["bp"] if i > 0 else fzero
            ET = work.tile([128, 128], F32, tag="ET")
            nc.scalar.activation(ET[:], pB[:, 384:512], AF.Exp,
                                 scale=-1.0 / 16.0, bias=bn[:])
            EnT = work.tile([128, 128], F32, tag="EnT")
            nc.scalar.activation(EnT[:], pB[:, 384:512], AF.Exp,
                                 scale=1.0 / 16.0, bias=bp[:])
            qt_sb = work.tile([128, 128], BF16, tag="qt")
            nc.vector.tensor_tensor(qt_sb[:], pA[:, 0:128], ET[:],
                                    mybir.AluOpType.mult)
            st["qt"] = qt_sb
            kt_sb = work.tile([128, 128], BF16, tag="kt")
            nc.vector.tensor_tensor(kt_sb[:], pA[:, 128:256], EnT[:],
                                    mybir.AluOpType.mult)
            st["kt"] = kt_sb
            # ktm evict late in the ACT queue: nothing reads it until the
            # next iteration's state-mm
            if 0 <= m1 < NCH and S[m1].get("trt") is not None:
                ktm = work.tile([128, 128], BF16, tag="ktm")
                nc.scalar.copy(ktm[:], S[m1]["trt"][:])
                S[m1]["ktm"] = ktm
            # silu gate = u * 1/(1+exp(-u)), all SBUF operands
            eg = wide.tile([128, DV], F32, tag="eg")
            nc.scalar.activation(eg[:], u_sb[:], AF.Exp, scale=-1.0)
            ug = wide.tile([128, DV], F32, tag="ug")
            nc.gpsimd.tensor_scalar_add(ug[:], eg[:], 1.0)
            sg = wide.tile([128, DV], F32, tag="sg")
            nc.vector.reciprocal(sg[:], ug[:])
            gate = wide.tile([128, DV], F32, tag="gate")
            nc.gpsimd.tensor_tensor(gate[:], u_sb[:], sg[:],
                                    mybir.AluOpType.mult)
            st["gate"] = gate
            if i < NCH - 1:
                bn = work.tile([128, 1], F32, tag="bn")
                nc.vector.tensor_scalar_mul(bn[:], wps[:, 256:257],
                                            -1.0 / 16.0)
                st["bn"] = bn
                bp = work.tile([128, 1], F32, tag="bp")
                nc.vector.tensor_scalar_mul(bp[:], wps[:, 256:257],
                                            1.0 / 16.0)
                st["bp"] = bp

        if 0 <= m1 < NCH:
            st1 = S[m1]
            # ---- D2: oT[dv,t] = v^T at (+ W^T q~T), dv halves ------------
            for half in range(2):
                ocols = slice(128 + half * 128, 256 + half * 128)
                vcols = slice(half * 128, 128 + half * 128)
                mm(st1["ao"][:, ocols], lhsT=st1["v"][:, vcols],
                   rhs=st1["atm"][:],
                   start=(half == 0), stop=False, skip_group_check=True)
                if m1 > 0:
                    mm(st1["ao"][:, ocols], lhsT=S[m2]["w"][:, vcols],
                       rhs=st1["qt"][:],
                       start=False, stop=False, skip_group_check=True)
            og = wide.tile([128, DV], BF16, tag="og")
            nc.vector.tensor_tensor(og[:], st1["ao"][:, 128:384],
                                    st1["gate"][:], mybir.AluOpType.mult)
            st1["og"] = og
            sq = wide.tile([128, DV], BF16, tag="sq")
            nc.scalar.square(sq[:], st1["ao"][:, 128:384])
            st1["sq"] = sq

        if 0 <= m2 < NCH:
            st2 = S[m2]
            # ---- C2: final projection + rms chain ------------------------
            fns = []
            for nb in range(2):
                cols = slice(nb * 512, (nb + 1) * 512)
                if m2 == NCH - 1:
                    # last chunk: steal the now-idle projection banks so the
                    # fin mms don't WAR-wait on chunk 6's osb evictions
                    pool, tg = (pjB, "pb") if nb == 0 else (pjA, "pa")
                    fn = pool.tile([128, 512], F32, tag=tg)
                else:
                    fn = finp.tile([128, 512], F32, tag="fin")
                mm(fn[:], lhsT=st2["og"][:, 0:128], rhs=wout_sb[:, 0, cols],
                   start=True, stop=False)
                mm(fn[:], lhsT=st2["og"][:, 128:256], rhs=wout_sb[:, 1, cols],
                   start=False, stop=True)
                fns.append(fn)
            ao2 = st2["ao"]
            for half in range(2):
                scols = slice(half * 128, 128 + half * 128)
                mm(ao2[:, 384:385], lhsT=st2["sq"][:, scols], rhs=inv_dv[:],
                   start=False, stop=False, skip_group_check=True)
            s_sb = work.tile([128, 1], F32, tag="s")
            nc.scalar.activation(s_sb[:], ao2[:, 384:385], AF.Ln,
                                 bias=eps_sb[:])
            r_sb = work.tile([128, 1], F32, tag="r")
            nc.scalar.activation(r_sb[:], s_sb[:], AF.Exp, scale=-0.5)
            # fused deferred RMS scale on the fin psum evictions
            osb = osbp.tile([128, D], BF16, tag="osb")
            nc.scalar.activation(osb[:, 0:512], fns[0][:], AF.Copy,
                                 scale=r_sb[:])
            nc.vector.tensor_scalar_mul(osb[:, 512:1024], fns[1][:], r_sb[:])

        if 0 <= m2 < NCH:
            rows = slice(m2 * C, (m2 + 1) * C)
            if m2 == NCH - 1:
                nc.sync.dma_start(out=out[rows, 0:512], in_=osb[:, 0:512])
                nc.sync.dma_start(out=out[rows, 512:1024],
                                  in_=osb[:, 512:1024])
            else:
                nc.sync.dma_start(out=out[rows, :], in_=osb[:])


def _build_nc():
    _patch_act_tables()
    nc = bacc.Bacc("TRN2", target_bir_lowering=False, debug=False, num_devices=8)
    ap = {
        "xT": nc.dram_tensor("xT", [128, NK, N], BF16, kind="ExternalInput").ap(),
        "x0z": nc.dram_tensor("x0z", [128, NK * 128 + NK * 128 + 128],
                              BF16, kind="ExternalInput").ap(),
        "wqk": nc.dram_tensor("wqk", [128, NK, 256], BF16,
                              kind="ExternalInput").ap(),
        "wgv": nc.dram_tensor("wgv", [128, NK, 512], BF16,
                              kind="ExternalInput").ap(),
        "woutT": nc.dram_tensor("woutT", [128, 2, D], BF16,
                                kind="ExternalInput").ap(),
        "lmask": nc.dram_tensor("lmask", [128, 128], F32,
                                kind="ExternalInput").ap(),
        "identb": nc.dram_tensor("identb", [128, 128], BF16,
                                 kind="ExternalInput").ap(),
        "out": nc.dram_tensor("out", [N, D], BF16, kind="ExternalOutput").ap(),
    }
    with tile.TileContext(nc) as tc:
        with ExitStack() as ctx:
            _emit_kernel(ctx, tc, ap)
    nc.compile()
    return nc


def kernel(x, Wq, Wk, Wv, Wg, Wgk1, Wgk2, bgk2, Wout, rms_w):
    global LAST_RESULTS
    import ml_dtypes
    bf16 = ml_dtypes.bfloat16

    x = np.asarray(x, np.float32)
    Wz = np.asarray(Wgk1, np.float32) @ np.asarray(Wgk2, np.float32)
    L = np.triu(np.ones((C, C), np.float32))
    I128b = np.eye(128, dtype=bf16)

    in_maps = []
    for core in range(8):
        b, h = core // H, core % H
        xTb = np.ascontiguousarray(
            x[b].T.astype(bf16).reshape(NK, 128, N).transpose(1, 0, 2))
        wz_h = np.ascontiguousarray(
            Wz[:, h * DK:(h + 1) * DK].astype(bf16)
            .reshape(NK, 128, 128).transpose(1, 0, 2))
        wqk_h = np.ascontiguousarray(np.concatenate(
            [Wq[:, h * DK:(h + 1) * DK], Wk[:, h * DK:(h + 1) * DK]],
            axis=1).astype(bf16).reshape(NK, 128, 256).transpose(1, 0, 2))
        wgv_h = np.ascontiguousarray(np.concatenate(
            [Wg[:, h * DV:(h + 1) * DV], Wv[:, h * DV:(h + 1) * DV]],
            axis=1).astype(bf16).reshape(NK, 128, 512).transpose(1, 0, 2))
        woutP = np.ascontiguousarray(
            (np.asarray(rms_w, np.float32)[:, None]
             * np.asarray(Wout, np.float32)[h * DV:(h + 1) * DV]).astype(bf16)
            .reshape(2, 128, D).transpose(1, 0, 2))
        in_maps.append({
            "xT": xTb,
            "x0z": np.ascontiguousarray(np.concatenate([
                xTb[:, :, 0:C].reshape(128, NK * C),
                wz_h.reshape(128, NK * 128),
                np.concatenate([np.asarray(bgk2, np.float32)
                                [h * DK:(h + 1) * DK][None, :],
                                np.zeros((127, 128), np.float32)],
                               axis=0).astype(bf16)], axis=1)),
            "wqk": wqk_h,
            "wgv": wgv_h,
            "woutT": woutP,
            "lmask": L,
            "identb": I128b,
        })

    global LAST_NC
    nc = _build_nc()
    LAST_NC = nc
    trace = os.environ.get("BASSGLA_TRACE", "0") == "1"
    res = run_bass_kernel_spmd(nc, in_maps, list(range(8)), trace=trace)
    LAST_RESULTS = res

    out = np.zeros((B, N, D), np.float32)
    for core in range(8):
        out[core // H] += np.asarray(res.results[core]["out"], np.float32)
    return out
